# revision 6
# baseline (speedup 1.0000x reference)
"""Causal depthwise conv (kernel_size=4) on 8 TRN2 NeuronCores.

Problem: x (4, 4096, 16, 128) f32, weight (4, 16, 128) f32,
out[b,t,h,d] = sum_k weight[k,h,d] * x[b,t-k,h,d]   (zero-pad t<0).

Sharding: tensor-parallel over heads — core c owns heads [2c, 2c+2).
Host transposes each core's slice to d-major layout so that on-device the
partition dim is d (128) and the free dim is t. Then weight[k,h,:] is a
per-partition scalar and the whole conv per (h, b) stream is:

    acc = w0 * x                          (ScalarE activation, scale=w0)
    acc = (x >> k) * wk + acc, k=1..3     (fused scalar_tensor_tensor)

The 3 STT taps are split between VectorE (cols [0, SPLIT)) and GpSimd
(cols [SPLIT, 4096)) which run concurrently. Each stream ships with 3 zero
columns prepended (causal pad), so taps never cross stream boundaries.
"""

import numpy as np

import concourse.mybir as mybir
from concourse import bacc, tile
from concourse.bass_utils import run_bass_kernel_spmd

BATCH, SEQ, N_HEADS, D_HEAD = 4, 4096, 16, 128
KERNEL = 4
PAD = KERNEL - 1
N_CORES = 8
H_PER_CORE = N_HEADS // N_CORES          # 2
N_STREAMS = H_PER_CORE * BATCH           # 8 per core; stream j = hl*BATCH + b
SPLIT = 2784                             # VectorE columns; GpSimd gets the rest

F32 = mybir.dt.float32

PROFILE = False          # set by test.py; adds a profiled run
TRACE_KWARGS = {}
last_exec_time_ns = None
last_results = None


def _build_module(chain: bool = False, repeats: int = 1):
    """chain=True builds the timing variant: out has the same shape as x
    (pad columns written as zeros) so outputs can feed back as inputs for
    device-resident repeated-execution timing. repeats>1 runs the whole
    kernel body that many times inside one NEFF (timing only)."""
    nc = bacc.Bacc(
        "TRN2",
        target_bir_lowering=False,
        debug=False,
        num_devices=N_CORES,
        enable_asserts=False,
    )
    out_t = SEQ + PAD if chain else SEQ
    x = nc.dram_tensor("x", [D_HEAD, N_STREAMS, SEQ + PAD], F32, kind="ExternalInput").ap()
    w = nc.dram_tensor("w", [D_HEAD, H_PER_CORE * KERNEL], F32, kind="ExternalInput").ap()
    out = nc.dram_tensor("out", [D_HEAD, N_STREAMS, out_t], F32, kind="ExternalOutput").ap()
    pad_off = PAD if chain else 0

    with tile.TileContext(nc) as tc:
        with (
            tc.tile_pool(name="wp", bufs=1) as wp,
            tc.tile_pool(name="xp", bufs=4) as xp,
            tc.tile_pool(name="dp", bufs=3) as dp,
        ):
            wt = wp.tile([D_HEAD, H_PER_CORE * KERNEL], F32)
            nc.sync.dma_start(out=wt, in_=w)
            if chain:
                zt = wp.tile([D_HEAD, N_STREAMS * PAD], F32)
                nc.vector.memset(zt, 0.0)
                nc.sync.dma_start(
                    out=out[:, :, 0:PAD], in_=zt[:, :].rearrange("p (j q) -> p j q", q=PAD)
                )
            for _r in range(repeats):
                for j in range(N_STREAMS):
                    hl = j // BATCH
                    X = xp.tile([D_HEAD, SEQ + PAD], F32, tag="x")
                    nc.sync.dma_start(out=X, in_=x[:, j, :])
                    acc = dp.tile([D_HEAD, SEQ], F32, tag="acc")
                    w0 = wt[:, hl * KERNEL : hl * KERNEL + 1]
                    nc.scalar.activation(
                        acc, X[:, PAD : PAD + SEQ],
                        mybir.ActivationFunctionType.Copy, scale=w0,
                    )
                    for k in range(1, KERNEL):
                        wk = wt[:, hl * KERNEL + k : hl * KERNEL + k + 1]
                        nc.vector.scalar_tensor_tensor(
                            acc, X[:, PAD - k : PAD + SEQ - k], wk, acc,
                            mybir.AluOpType.mult, mybir.AluOpType.add,
                        )
                    nc.sync.dma_start(out=out[:, j, pad_off : pad_off + SEQ], in_=acc)
    nc.compile()
    return nc


_module = None


def _get_module():
    global _module
    if _module is None:
        _module = _build_module()
    return _module


def _shard_inputs(x: np.ndarray, weight: np.ndarray):
    in_maps = []
    for c in range(N_CORES):
        h0 = c * H_PER_CORE
        xs = x[:, :, h0 : h0 + H_PER_CORE, :]            # (B, T, HL, D)
        xt = np.ascontiguousarray(xs.transpose(3, 2, 0, 1))  # (D, HL, B, T)
        xin = np.zeros((D_HEAD, N_STREAMS, SEQ + PAD), dtype=np.float32)
        xin[:, :, PAD:] = xt.reshape(D_HEAD, N_STREAMS, SEQ)
        ws = weight[:, h0 : h0 + H_PER_CORE, :]          # (K, HL, D)
        warr = np.ascontiguousarray(ws.transpose(2, 1, 0)).reshape(D_HEAD, H_PER_CORE * KERNEL)
        in_maps.append({"x": xin, "w": warr.astype(np.float32)})
    return in_maps


def _unshard(results) -> np.ndarray:
    out = np.empty((BATCH, SEQ, N_HEADS, D_HEAD), dtype=np.float32)
    for c in range(N_CORES):
        h0 = c * H_PER_CORE
        o = results[c]["out"].reshape(D_HEAD, H_PER_CORE, BATCH, SEQ)
        out[:, :, h0 : h0 + H_PER_CORE, :] = o.transpose(2, 3, 1, 0)
    return out


def kernel(x: np.ndarray, weight: np.ndarray) -> np.ndarray:
    global last_exec_time_ns, last_results
    x = np.asarray(x, dtype=np.float32)
    weight = np.asarray(weight, dtype=np.float32)
    nc = _get_module()
    in_maps = _shard_inputs(x, weight)
    res = run_bass_kernel_spmd(
        nc, in_maps, list(range(N_CORES)), trace=PROFILE, **TRACE_KWARGS
    )
    last_exec_time_ns = res.exec_time_ns
    last_results = res
    return _unshard(res.results)


# revision 7
# speedup vs baseline: 1.0115x; 1.0115x over previous
"""Causal depthwise conv (kernel_size=4) on 8 TRN2 NeuronCores.

Problem: x (4, 4096, 16, 128) f32, weight (4, 16, 128) f32,
out[b,t,h,d] = sum_k weight[k,h,d] * x[b,t-k,h,d]   (zero-pad t<0).

Sharding: tensor-parallel over heads — core c owns heads [2c, 2c+2).
Host transposes each core's slice to d-major layout so that on-device the
partition dim is d (128) and the free dim is t. Then weight[k,h,:] is a
per-partition scalar and the whole conv per (h, b) stream is:

    acc = w0 * x                          (ScalarE activation, scale=w0)
    acc = (x >> k) * wk + acc, k=1..3     (fused scalar_tensor_tensor)

The 3 STT taps are split between VectorE (cols [0, SPLIT)) and GpSimd
(cols [SPLIT, 4096)) which run concurrently. Each stream ships with 3 zero
columns prepended (causal pad), so taps never cross stream boundaries.
"""

import numpy as np

import concourse.mybir as mybir
from concourse import bacc, tile
from concourse.bass_utils import run_bass_kernel_spmd

BATCH, SEQ, N_HEADS, D_HEAD = 4, 4096, 16, 128
KERNEL = 4
PAD = KERNEL - 1
N_CORES = 8
H_PER_CORE = N_HEADS // N_CORES          # 2
N_STREAMS = H_PER_CORE * BATCH           # 8 per core; stream j = hl*BATCH + b
SPLIT = 2784                             # VectorE columns; GpSimd gets the rest

F32 = mybir.dt.float32

PROFILE = False          # set by test.py; adds a profiled run
TRACE_KWARGS = {}
last_exec_time_ns = None
last_results = None


def _build_module(chain: bool = False, repeats: int = 1):
    """chain=True builds the timing variant: out has the same shape as x
    (pad columns written as zeros) so outputs can feed back as inputs for
    device-resident repeated-execution timing. repeats>1 runs the whole
    kernel body that many times inside one NEFF (timing only)."""
    nc = bacc.Bacc(
        "TRN2",
        target_bir_lowering=False,
        debug=False,
        num_devices=N_CORES,
        enable_asserts=False,
    )
    out_t = SEQ + PAD if chain else SEQ
    x = nc.dram_tensor("x", [D_HEAD, N_STREAMS, SEQ + PAD], F32, kind="ExternalInput").ap()
    w = nc.dram_tensor("w", [D_HEAD, H_PER_CORE * KERNEL], F32, kind="ExternalInput").ap()
    out = nc.dram_tensor("out", [D_HEAD, N_STREAMS, out_t], F32, kind="ExternalOutput").ap()
    pad_off = PAD if chain else 0

    with tile.TileContext(nc) as tc:
        with (
            tc.tile_pool(name="wp", bufs=1) as wp,
            tc.tile_pool(name="xp", bufs=6) as xp,
            tc.tile_pool(name="dp", bufs=4) as dp,
        ):
            wt = wp.tile([D_HEAD, H_PER_CORE * KERNEL], F32)
            nc.sync.dma_start(out=wt, in_=w)
            if chain:
                zt = wp.tile([D_HEAD, N_STREAMS * PAD], F32)
                nc.vector.memset(zt, 0.0)
                nc.sync.dma_start(
                    out=out[:, :, 0:PAD], in_=zt[:, :].rearrange("p (j q) -> p j q", q=PAD)
                )
            for _r in range(repeats):
                for j in range(N_STREAMS):
                    hl = j // BATCH
                    X = xp.tile([D_HEAD, SEQ + PAD], F32, tag="x")
                    nc.sync.dma_start(out=X, in_=x[:, j, :])
                    acc = dp.tile([D_HEAD, SEQ], F32, tag="acc")
                    w0 = wt[:, hl * KERNEL : hl * KERNEL + 1]
                    nc.scalar.activation(
                        acc, X[:, PAD : PAD + SEQ],
                        mybir.ActivationFunctionType.Copy, scale=w0,
                    )
                    for k in range(1, KERNEL):
                        wk = wt[:, hl * KERNEL + k : hl * KERNEL + k + 1]
                        nc.vector.scalar_tensor_tensor(
                            acc, X[:, PAD - k : PAD + SEQ - k], wk, acc,
                            mybir.AluOpType.mult, mybir.AluOpType.add,
                        )
                    nc.sync.dma_start(out=out[:, j, pad_off : pad_off + SEQ], in_=acc)
    nc.compile()
    return nc


_module = None


def _get_module():
    global _module
    if _module is None:
        _module = _build_module()
    return _module


def _shard_inputs(x: np.ndarray, weight: np.ndarray):
    in_maps = []
    for c in range(N_CORES):
        h0 = c * H_PER_CORE
        xs = x[:, :, h0 : h0 + H_PER_CORE, :]            # (B, T, HL, D)
        xt = np.ascontiguousarray(xs.transpose(3, 2, 0, 1))  # (D, HL, B, T)
        xin = np.zeros((D_HEAD, N_STREAMS, SEQ + PAD), dtype=np.float32)
        xin[:, :, PAD:] = xt.reshape(D_HEAD, N_STREAMS, SEQ)
        ws = weight[:, h0 : h0 + H_PER_CORE, :]          # (K, HL, D)
        warr = np.ascontiguousarray(ws.transpose(2, 1, 0)).reshape(D_HEAD, H_PER_CORE * KERNEL)
        in_maps.append({"x": xin, "w": warr.astype(np.float32)})
    return in_maps


def _unshard(results) -> np.ndarray:
    out = np.empty((BATCH, SEQ, N_HEADS, D_HEAD), dtype=np.float32)
    for c in range(N_CORES):
        h0 = c * H_PER_CORE
        o = results[c]["out"].reshape(D_HEAD, H_PER_CORE, BATCH, SEQ)
        out[:, :, h0 : h0 + H_PER_CORE, :] = o.transpose(2, 3, 1, 0)
    return out


def kernel(x: np.ndarray, weight: np.ndarray) -> np.ndarray:
    global last_exec_time_ns, last_results
    x = np.asarray(x, dtype=np.float32)
    weight = np.asarray(weight, dtype=np.float32)
    nc = _get_module()
    in_maps = _shard_inputs(x, weight)
    res = run_bass_kernel_spmd(
        nc, in_maps, list(range(N_CORES)), trace=PROFILE, **TRACE_KWARGS
    )
    last_exec_time_ns = res.exec_time_ns
    last_results = res
    return _unshard(res.results)
